# revision 1
# baseline (speedup 1.0000x reference)
"""Trainium2 Bass kernel for GridSmoother: per-batch SPD grid-Laplacian solve.

System: L = I + Dx^T Wx Dx + Dy^T Wy Dy over a 48x64 grid, solved for 16
channels per batch, B=4 batches.  lambda(L) in [1, 1+2*max_node(sum w)] --
tiny condition number, so a fixed-coefficient Chebyshev iteration on the
5-point stencil converges at ~0.5x error per iteration.

Sharding: batch b -> cores {2b, 2b+1}, each core owns 8 channels.
Per-core data layout (SBUF tile [128 partitions, 194 free]):
  partition p = (c_local//4)*64 + w      (c_hi in {0,1}, w in 0..63)
  free      f = 1 + (c_local%4)*48 + h   (c_lo in {0..3}, h in 0..47)
  f=0 and f=193 are zero guard columns.
Vertical (h+-1) neighbor access = free-dim offset reads (guards + zeroed
boundary weights make wraps harmless).  Horizontal (w+-1) = partition shifts
done on the TensorEngine with block-diagonal +-1 shift matrices, accumulated
in PSUM together with the diagonal and vertical terms (5 matmuls -> P = A*u).
"""

import numpy as np
import sys

sys.path.insert(0, "/opt/trn_rl_repo")

import concourse.bass as bass
from concourse import mybir
from concourse.bass_utils import run_bass_kernel_spmd

B, C, H, W = 4, 16, 48, 64
NCORE = 8
CPC = C // 2          # channels per core = 8
FD = 194              # free dim incl. 2 guards
FDA = 192             # active free size
NBLK = 5              # wxz, wxzUP, wyz, wyzUP, diag

F32 = mybir.dt.float32

_COMPILED = {}


def _planes(wx, wy):
    """Per-batch weight planes in (h, w) image space."""
    wxz = wx.copy()
    wxz[:, -1] = 0.0
    wyz = wy.copy()
    wyz[-1, :] = 0.0
    wxzUP = np.zeros_like(wxz)
    wxzUP[:, 1:] = wxz[:, :-1]
    wyzUP = np.zeros_like(wyz)
    wyzUP[1:, :] = wyz[:-1, :]
    diag = 1.0 + wxz + wxzUP + wyz + wyzUP
    return wxz, wxzUP, wyz, wyzUP, diag


def _plane2core(plane_hw):
    """[48,64] (h,w) plane -> [128,194] core layout with zero guards."""
    out = np.zeros((128, FD), dtype=np.float32)
    t = plane_hw.T  # [w, h] = [64, 48]
    out[:, 1:193] = np.tile(t, (2, 4))
    return out


def _b2core(ae_chans):
    """[8,48,64] -> [128,192]."""
    a = ae_chans.reshape(2, 4, H, W)
    a = np.transpose(a, (0, 3, 1, 2))  # [c_hi, w, c_lo, h]
    return np.ascontiguousarray(a.reshape(128, FDA), dtype=np.float32)


def _core2out(xt):
    """[128,192] -> [8,48,64]."""
    a = xt.reshape(2, W, 4, H)
    a = np.transpose(a, (0, 2, 3, 1))  # [c_hi, c_lo, h, w]
    return a.reshape(CPC, H, W)


def _shift_mats():
    """lhsT matrices [128,128]: I(+1), SupN(-1 at k=m-1), SdnN(-1 at k=m+1),
    IN(-I). Block-diagonal over the two 64-partition halves."""
    ipos = np.eye(128, dtype=np.float32)
    sup = np.zeros((128, 128), dtype=np.float32)
    sdn = np.zeros((128, 128), dtype=np.float32)
    for m in range(128):
        if m % 64 != 0:
            sup[m - 1, m] = -1.0
        if m % 64 != 63:
            sdn[m + 1, m] = -1.0
    ineg = -ipos
    return np.concatenate([ipos, sup, sdn, ineg], axis=1)  # [128, 512]


def _cheby_coeffs(lam_max, n_iter):
    """Returns per-iteration (gamma_k, c_next_k) for the scaled-direction
    Chebyshev recurrence:
        x += gamma_k * u ; r -= gamma_k * A u ; u = c_{k+1} * u + r
    """
    lmin = 1.0
    theta = (lam_max + lmin) / 2.0
    delta = (lam_max - lmin) / 2.0
    sigma1 = theta / delta
    gammas, cnexts = [], []
    gamma = 1.0 / theta
    rho = 1.0 / sigma1
    for _ in range(n_iter):
        rho_next = 1.0 / (2.0 * sigma1 - rho)
        c_next = rho * gamma * delta / 2.0
        gamma_next = 2.0 * rho_next / delta
        gammas.append(gamma)
        cnexts.append(c_next)
        rho, gamma = rho_next, gamma_next
    return gammas, cnexts


N_ITER = 20


def _build(lam_max, n_iter):
    """Raw Bass program (explicit semaphores; every instruction carries at
    most one wait -- the walrus codegen on this path rejects multi-wait
    sync_info)."""
    nc = bass.Bass("TRN2", target_bir_lowering=False, debug=False,
                   num_devices=NCORE, detect_race_conditions=False)
    bt_d = nc.dram_tensor("bt", [128, FDA], F32, kind="ExternalInput").ap()
    wcat_d = nc.dram_tensor("wcat", [128, NBLK * FD], F32,
                            kind="ExternalInput").ap()
    smats_d = nc.dram_tensor("smats", [128, 512], F32,
                             kind="ExternalInput").ap()
    xout_d = nc.dram_tensor("xout", [128, FDA], F32,
                            kind="ExternalOutput").ap()

    gammas, cnexts = _cheby_coeffs(lam_max, n_iter)
    theta = (lam_max + 1.0) / 2.0

    wcat = nc.alloc_sbuf_tensor("wcat_s", [128, NBLK * FD], F32).ap()
    smats = nc.alloc_sbuf_tensor("smats_s", [128, 512], F32).ap()
    btile = nc.alloc_sbuf_tensor("btile_s", [128, FDA], F32).ap()
    r = nc.alloc_sbuf_tensor("r_s", [128, FD], F32).ap()
    u = nc.alloc_sbuf_tensor("u_s", [128, FD], F32).ap()
    x = nc.alloc_sbuf_tensor("x_s", [128, FD], F32).ap()
    pc = nc.alloc_sbuf_tensor("pc_s", [128, NBLK * FD], F32).ap()
    P = nc.alloc_psum_tensor("P_s", [128, FDA], F32).ap()

    mI = smats[:, 0:128]
    mSup = smats[:, 128:256]
    mSdn = smats[:, 256:384]
    mIN = smats[:, 384:512]

    u_b = u.rearrange("p (o f) -> p o f", o=1).broadcast_to([128, NBLK, FD])
    w_b = wcat.rearrange("p (o f) -> p o f", o=NBLK)
    pc_b = pc.rearrange("p (o f) -> p o f", o=NBLK)

    dma_sem = nc.alloc_semaphore("dma_sem")
    dve_sem = nc.alloc_semaphore("dve_sem")   # counts pc-ready TTs
    pe_sem = nc.alloc_semaphore("pe_sem")     # counts matmuls
    gp_sem = nc.alloc_semaphore("gp_sem")     # x memset done
    out_sem = nc.alloc_semaphore("out_sem")   # final x ready

    with nc.Block() as block:

        @block.gpsimd
        def _(gp):
            gp.dma_start(wcat, wcat_d).then_inc(dma_sem, 16)
            gp.dma_start(smats, smats_d).then_inc(dma_sem, 16)
            gp.dma_start(btile, bt_d).then_inc(dma_sem, 16)
            gp.memset(x, 0.0).then_inc(gp_sem, 1)
            gp.wait_ge(out_sem, 1)
            gp.dma_start(xout_d, x[:, 1:193]).then_inc(dma_sem, 16)
            gp.wait_ge(dma_sem, 64)

        @block.tensor
        def _(pe):
            pe.wait_ge(dma_sem, 48)  # all inputs loaded
            for k in range(n_iter - 1):
                pe.wait_ge(dve_sem, 2 * k + 1)
                pe.matmul(P, mSup, pc[:, 0 * FD + 1:0 * FD + 193],
                          start=True, stop=False).then_inc(pe_sem, 1)
                pe.matmul(P, mSdn, pc[:, 1 * FD + 1:1 * FD + 193],
                          start=False, stop=False).then_inc(pe_sem, 1)
                pe.wait_ge(dve_sem, 2 * k + 2)
                pe.matmul(P, mI, pc[:, 4 * FD + 1:4 * FD + 193],
                          start=False, stop=False).then_inc(pe_sem, 1)
                pe.matmul(P, mIN, pc[:, 2 * FD + 0:2 * FD + 192],
                          start=False, stop=False).then_inc(pe_sem, 1)
                pe.matmul(P, mIN, pc[:, 3 * FD + 2:3 * FD + 194],
                          start=False, stop=True).then_inc(pe_sem, 1)

        @block.vector
        def _(v):
            v.wait_ge(dma_sem, 48)  # btile (and everything before) loaded
            v.memset(r, 0.0)
            v.tensor_copy(r[:, 1:193], btile)
            v.tensor_scalar_mul(u, r, 1.0 / theta)
            v.wait_ge(gp_sem, 1)    # x memset done
            for k in range(n_iter):
                g = float(gammas[k])
                if k == n_iter - 1:
                    v.scalar_tensor_tensor(
                        x, u, g, x,
                        mybir.AluOpType.mult,
                        mybir.AluOpType.add).then_inc(out_sem, 1)
                    break
                c = float(cnexts[k])
                u_b2 = u.rearrange("p (o f) -> p o f", o=1).broadcast_to(
                    [128, 2, FD])
                u_b3 = u.rearrange("p (o f) -> p o f", o=1).broadcast_to(
                    [128, 3, FD])
                v.tensor_tensor(
                    pc[:, 0:2 * FD].rearrange("p (o f) -> p o f", o=2),
                    wcat[:, 0:2 * FD].rearrange("p (o f) -> p o f", o=2),
                    u_b2, mybir.AluOpType.mult).then_inc(dve_sem, 1)
                v.tensor_tensor(
                    pc[:, 2 * FD:5 * FD].rearrange("p (o f) -> p o f", o=3),
                    wcat[:, 2 * FD:5 * FD].rearrange("p (o f) -> p o f", o=3),
                    u_b3, mybir.AluOpType.mult).then_inc(dve_sem, 1)
                # x += gamma * u (runs while PE computes A u)
                v.scalar_tensor_tensor(x, u, g, x,
                                       mybir.AluOpType.mult,
                                       mybir.AluOpType.add)
                v.wait_ge(pe_sem, 5 * (k + 1))
                # r -= gamma * P
                v.scalar_tensor_tensor(r[:, 1:193], P, -g, r[:, 1:193],
                                       mybir.AluOpType.mult,
                                       mybir.AluOpType.add)
                # u = c_next * u + r
                v.scalar_tensor_tensor(u, u, c, r,
                                       mybir.AluOpType.mult,
                                       mybir.AluOpType.add)

    return nc


def kernel(ae: np.ndarray, wxwy: np.ndarray) -> np.ndarray:
    ae = np.asarray(ae, dtype=np.float32)
    wxwy = np.asarray(wxwy, dtype=np.float32)

    # ---- host prep: per-core shards -------------------------------------
    smats = _shift_mats()
    in_maps = []
    lam_max = 0.0
    wcats = []
    for b in range(B):
        wxz, wxzUP, wyz, wyzUP, diag = _planes(wxwy[b, 0], wxwy[b, 1])
        inc = wxz + wxzUP + wyz + wyzUP
        lam_max = max(lam_max, 1.0 + 2.0 * float(inc.max()))
        wcats.append(np.concatenate(
            [_plane2core(p) for p in (wxz, wxzUP, wyz, wyzUP, diag)], axis=1))
    # round lam_max up a touch for a safe, cache-friendly constant
    lam_max = float(np.ceil(lam_max * 64.0) / 64.0)

    for core in range(NCORE):
        b, half = core // 2, core % 2
        bt = _b2core(ae[b, half * CPC:(half + 1) * CPC])
        in_maps.append({"bt": bt, "wcat": wcats[b], "smats": smats})

    key = (lam_max, N_ITER)
    if key not in _COMPILED:
        _COMPILED[key] = _build(lam_max, N_ITER)
    nc = _COMPILED[key]

    global _LAST_BUILD
    _LAST_BUILD = (nc, in_maps)

    res = run_bass_kernel_spmd(nc, in_maps, list(range(NCORE)))

    out = np.empty((B, C, H, W), dtype=np.float32)
    for core in range(NCORE):
        b, half = core // 2, core % 2
        out[b, half * CPC:(half + 1) * CPC] = _core2out(
            res.results[core]["xout"])
    return out



# revision 4
# speedup vs baseline: 3.0728x; 3.0728x over previous
"""Trainium2 Bass kernel for GridSmoother: per-batch SPD grid-Laplacian solve.

System: L = I + Dx^T Wx Dx + Dy^T Wy Dy over a 48x64 grid, 16 channels per
batch, B=4.  lambda(L) in [1, 9] (Gershgorin, weights < 1), so a
fixed-coefficient Chebyshev iteration on the 5-point stencil converges at
~0.5x error per iteration; K=12 iterations reach ~5e-4 relative error,
far inside the 2e-2 gate.

This problem is wall-clock-dominated by host->device dispatch through the
PJRT relay, not device compute (~0.2 ms of DVE work).  The kernel is
therefore built to minimize per-call overhead:
  * single NeuronCore (core-count showed no win at fixed volume, and the
    1-core jit path skips shard_map),
  * fp16 I/O: ae rhs 393KB + compact weight planes 123KB in, 393KB out,
  * no TensorEngine/PSUM: horizontal (w+-1) neighbor terms use
    partition-shifted SBUF->SBUF DMA copies of u; vertical (h+-1) terms
    use free-dim offset views; everything else is Vector-engine ops,
  * a persistent jax compilation cache so repeat calls skip the
    neuronxcc/BIR re-verification (~150-400 ms/call otherwise),
  * fixed lam_max=9.0 so the compiled program is input-independent.

Tile layout (8 tiles t = 2*b + half, half selects 8 of 16 channels):
  partition p = (c_local//4)*64 + w      (c_hi in {0,1}, w in 0..63)
  free      f = t*194 + 1 + (c_local%4)*48 + h
  f = t*194 and t*194+193 are zero guard columns.
Weight planes (host-derived, fp16, w-major compact [64, 4*5*48]):
  k=0: wxz   (* u[w+1] via DMA shift)    k=1: wxzUP (* u[w-1])
  k=2: wyzUP (* u[f-1] via offset view)  k=3: wyz   (* u[f+1])
  k=4: diag = 1 + wxz + wxzUP + wyz + wyzUP
Boundary weights are zeroed on host, so shift wrap-around terms vanish.
"""

import numpy as np
import sys

sys.path.insert(0, "/opt/trn_rl_repo")

import jax

jax.config.update("jax_compilation_cache_dir", "/tmp/jax_pcc")
jax.config.update("jax_persistent_cache_min_compile_time_secs", 0)
jax.config.update("jax_persistent_cache_min_entry_size_bytes", -1)

import concourse.bass as bass
from concourse import mybir
from concourse.bass_utils import run_bass_kernel_spmd

B, C, H, W = 4, 16, 48, 64
T = 8                 # tiles (b, half)
FD = 194              # per-tile free extent incl. 2 guards
FDA = 192             # active free size
WID = T * FD          # 1552
NPL = 5               # weight planes
LAM_MAX = 9.0         # Gershgorin bound: 1 + 2*4*max(w), w<1
N_ITER = 12

F32 = mybir.dt.float32
F16 = mybir.dt.float16

_COMPILED = {}


def _planes(wx, wy):
    """Per-batch [48,64] (h,w) planes in multiplication order
    (wxz, wxzUP, wyzUP, wyz, diag), boundaries zeroed."""
    wxz = wx.copy()
    wxz[:, -1] = 0.0
    wyz = wy.copy()
    wyz[-1, :] = 0.0
    wxzUP = np.zeros_like(wxz)
    wxzUP[:, 1:] = wxz[:, :-1]
    wyzUP = np.zeros_like(wyz)
    wyzUP[1:, :] = wyz[:-1, :]
    diag = 1.0 + wxz + wxzUP + wyz + wyzUP
    return wxz, wxzUP, wyzUP, wyz, diag


def _b2tile(ae_chans):
    """[8,48,64] -> [128,192] (p=(c_hi,w), f=(c_lo,h))."""
    a = ae_chans.reshape(2, 4, H, W)
    a = np.transpose(a, (0, 3, 1, 2))  # [c_hi, w, c_lo, h]
    return a.reshape(128, FDA)


def _tile2out(xt):
    """[128,192] -> [8,48,64]."""
    a = xt.reshape(2, W, 4, H)
    a = np.transpose(a, (0, 2, 3, 1))  # [c_hi, c_lo, h, w]
    return a.reshape(C // 2, H, W)


def _cheby_coeffs(lam_max, n_iter):
    """Per-iteration (gamma_k, c_next_k) for the scaled-direction Chebyshev
    recurrence: x += gamma_k*u ; r -= gamma_k*A u ; u = c_{k+1}*u + r."""
    lmin = 1.0
    theta = (lam_max + lmin) / 2.0
    delta = (lam_max - lmin) / 2.0
    sigma1 = theta / delta
    gammas, cnexts = [], []
    gamma = 1.0 / theta
    rho = 1.0 / sigma1
    for _ in range(n_iter):
        rho_next = 1.0 / (2.0 * sigma1 - rho)
        c_next = rho * gamma * delta / 2.0
        gamma_next = 2.0 * rho_next / delta
        gammas.append(gamma)
        cnexts.append(c_next)
        rho, gamma = rho_next, gamma_next
    return gammas, cnexts


def _build(n_iter):
    """Raw Bass program, single core, GPSIMD (DMA) + Vector engines only.
    Every instruction carries at most one wait (walrus codegen limit)."""
    nc = bass.Bass("TRN2", target_bir_lowering=False, debug=False,
                   num_devices=1, detect_race_conditions=False)
    bt_d = nc.dram_tensor("bt", [128, T * FDA], F16, kind="ExternalInput").ap()
    wc_d = nc.dram_tensor("wc", [64, B * NPL * H], F16,
                          kind="ExternalInput").ap()
    xo_d = nc.dram_tensor("xo", [128, T * FDA], F16,
                          kind="ExternalOutput").ap()

    gammas, cnexts = _cheby_coeffs(LAM_MAX, n_iter)
    theta = (LAM_MAX + 1.0) / 2.0

    s_bt = nc.alloc_sbuf_tensor("s_bt", [128, T * FDA], F16).ap()
    s_wc = nc.alloc_sbuf_tensor("s_wc", [64, B * NPL * H], F16).ap()
    s_xo = nc.alloc_sbuf_tensor("s_xo", [128, T * FDA], F16).ap()
    wpl = nc.alloc_sbuf_tensor("wpl", [128, NPL * WID], F32).ap()
    u = nc.alloc_sbuf_tensor("u", [128, WID], F32).ap()
    r = nc.alloc_sbuf_tensor("r", [128, WID], F32).ap()
    x = nc.alloc_sbuf_tensor("x", [128, WID], F32).ap()
    uup = nc.alloc_sbuf_tensor("uup", [128, WID], F32).ap()
    udn = nc.alloc_sbuf_tensor("udn", [128, WID], F32).ap()
    pd = nc.alloc_sbuf_tensor("pd", [128, WID], F32).ap()
    p0 = nc.alloc_sbuf_tensor("p0", [128, WID], F32).ap()
    p1 = nc.alloc_sbuf_tensor("p1", [128, WID], F32).ap()
    p2 = nc.alloc_sbuf_tensor("p2", [128, WID], F32).ap()
    p3 = nc.alloc_sbuf_tensor("p3", [128, WID], F32).ap()

    w0 = wpl[:, 0 * WID:1 * WID]
    w1 = wpl[:, 1 * WID:2 * WID]
    w2 = wpl[:, 2 * WID:3 * WID]
    w3 = wpl[:, 3 * WID:4 * WID]
    w4 = wpl[:, 4 * WID:5 * WID]

    dsem = nc.alloc_semaphore("dsem")   # input/output + wpl-dup DMA
    ssem = nc.alloc_semaphore("ssem")   # per-iter shift DMAs
    vsem = nc.alloc_semaphore("vsem")   # wpl rows 0:64 built
    usem = nc.alloc_semaphore("usem")   # u-ready count
    osem = nc.alloc_semaphore("osem")   # s_xo cast done

    MULT = mybir.AluOpType.mult
    ADD = mybir.AluOpType.add

    with nc.Block() as block:

        @block.gpsimd
        def _(gp):
            gp.dma_start(s_bt, bt_d).then_inc(dsem, 16)
            gp.dma_start(s_wc, wc_d).then_inc(dsem, 16)
            gp.wait_ge(vsem, 1)
            gp.dma_start(wpl[64:128, :], wpl[0:64, :]).then_inc(dsem, 16)
            for k in range(n_iter - 1):
                gp.wait_ge(usem, k + 1)
                gp.dma_start(uup[0:127, :], u[1:128, :]).then_inc(ssem, 16)
                gp.dma_start(udn[1:128, :], u[0:127, :]).then_inc(ssem, 16)
            gp.wait_ge(osem, 1)
            gp.dma_start(xo_d, s_xo).then_inc(dsem, 16)
            gp.wait_ge(dsem, 64)

        @block.vector
        def _(v):
            v.memset(uup, 0.0)
            v.memset(udn, 0.0)
            v.memset(p2, 0.0)
            v.memset(p3, 0.0)
            v.memset(r, 0.0)
            v.memset(wpl[0:64, :], 0.0)
            v.wait_ge(dsem, 32)  # both inputs in SBUF
            # scatter compact fp16 planes into guarded fp32 layout, rows 0:64
            scatter = []
            for k in range(NPL):
                for b in range(B):
                    src = s_wc[:, (b * NPL + k) * H:(b * NPL + k + 1) * H]
                    for half in range(2):
                        t = 2 * b + half
                        for cl in range(4):
                            off = k * WID + t * FD + 1 + cl * H
                            scatter.append(
                                v.tensor_copy(wpl[0:64, off:off + H], src))
            scatter[-1].then_inc(vsem, 1)
            # rhs placement: r active slices <- s_bt (fp16->fp32)
            for t in range(T):
                v.tensor_copy(r[:, t * FD + 1:t * FD + 193],
                              s_bt[:, t * FDA:(t + 1) * FDA])
            v.tensor_scalar_mul(u, r, 1.0 / theta).then_inc(usem, 1)
            v.wait_ge(dsem, 48)  # wpl rows 64:128 duplicated
            for k in range(n_iter):
                g = float(gammas[k])
                if k == 0:
                    v.tensor_scalar_mul(x, u, g)
                else:
                    v.scalar_tensor_tensor(x, u, g, x, MULT, ADD)
                if k == n_iter - 1:
                    break
                c = float(cnexts[k])
                v.tensor_tensor(pd, w4, u, MULT)
                v.tensor_tensor(p2[:, 1:WID], w2[:, 1:WID],
                                u[:, 0:WID - 1], MULT)
                v.tensor_tensor(p3[:, 0:WID - 1], w3[:, 0:WID - 1],
                                u[:, 1:WID], MULT)
                v.wait_ge(ssem, 32 * (k + 1))
                v.tensor_tensor(p0, w0, uup, MULT)
                v.tensor_tensor(p1, w1, udn, MULT)
                v.scalar_tensor_tensor(r, pd, -g, r, MULT, ADD)
                v.scalar_tensor_tensor(r, p0, g, r, MULT, ADD)
                v.scalar_tensor_tensor(r, p1, g, r, MULT, ADD)
                v.scalar_tensor_tensor(r, p2, g, r, MULT, ADD)
                v.scalar_tensor_tensor(r, p3, g, r, MULT, ADD)
                v.scalar_tensor_tensor(u, u, c, r, MULT, ADD).then_inc(
                    usem, 1)
            for t in range(T):
                cp = v.tensor_copy(s_xo[:, t * FDA:(t + 1) * FDA],
                                   x[:, t * FD + 1:t * FD + 193])
                if t == T - 1:
                    cp.then_inc(osem, 1)

    return nc


def _host_prep(ae, wxwy):
    bt = np.empty((128, T * FDA), dtype=np.float16)
    for t in range(T):
        b, half = t // 2, t % 2
        bt[:, t * FDA:(t + 1) * FDA] = _b2tile(
            ae[b, half * (C // 2):(half + 1) * (C // 2)]).astype(np.float16)
    wc = np.empty((64, B * NPL * H), dtype=np.float16)
    for b in range(B):
        planes = _planes(wxwy[b, 0], wxwy[b, 1])
        for k in range(NPL):
            # [48,64] (h,w) -> w-major [64,48]
            wc[:, (b * NPL + k) * H:(b * NPL + k + 1) * H] = \
                planes[k].T.astype(np.float16)
    return bt, wc


def kernel(ae: np.ndarray, wxwy: np.ndarray) -> np.ndarray:
    ae = np.asarray(ae, dtype=np.float32)
    wxwy = np.asarray(wxwy, dtype=np.float32)

    bt, wc = _host_prep(ae, wxwy)
    in_maps = [{"bt": bt, "wc": wc}]

    if N_ITER not in _COMPILED:
        _COMPILED[N_ITER] = _build(N_ITER)
    nc = _COMPILED[N_ITER]

    global _LAST_BUILD
    _LAST_BUILD = (nc, in_maps)

    res = run_bass_kernel_spmd(nc, in_maps, [0])

    xo = np.asarray(res.results[0]["xo"], dtype=np.float32)
    out = np.empty((B, C, H, W), dtype=np.float32)
    for t in range(T):
        b, half = t // 2, t % 2
        out[b, half * (C // 2):(half + 1) * (C // 2)] = _tile2out(
            xo[:, t * FDA:(t + 1) * FDA])
    return out


NCORE = 1  # cores used by _LAST_BUILD (test.py reads this)


# revision 5
# speedup vs baseline: 4.8001x; 1.5621x over previous
"""Trainium2 Bass kernel for GridSmoother: per-batch SPD grid-Laplacian solve.

System: L = I + Dx^T Wx Dx + Dy^T Wy Dy over a 48x64 grid, 16 channels per
batch, B=4.  lambda(L) in [1, 9] (Gershgorin, weights < 1), so a
fixed-coefficient Chebyshev iteration on the 5-point stencil converges at
~0.5x error per iteration; K=12 iterations reach ~5e-4 relative error,
far inside the 2e-2 gate.

This problem is wall-clock-dominated by host->device dispatch through the
PJRT relay, not device compute (~0.3 ms of simulated device time).  The
kernel is therefore built to minimize per-call overhead:
  * single NeuronCore (core count showed no win at fixed volume, and the
    1-core jit path skips shard_map),
  * fp16 I/O, one merged input buffer (ae rhs + compact weight planes,
    516KB) and one fp16 output (393KB),
  * no TensorEngine/PSUM: horizontal (w+-1) neighbor terms use
    partition-shifted SBUF->SBUF DMA copies of u; vertical (h+-1) terms
    use free-dim offset views; everything else is Vector-engine ops,
  * a persistent jax compilation cache plus a process-level cache of the
    loaded executable: re-creating the PJRT executable per call costs
    ~70 ms of NEFF reload on the device, so kernel() compiles/loads via
    bass_utils.run_bass_kernel_spmd on the first call and executes the
    cached executable (same custom-call binding) on repeat calls,
  * fixed lam_max=9.0 so the compiled program is input-independent.

Tile layout (8 tiles t = 2*b + half, half selects 8 of 16 channels):
  partition p = (c_local//4)*64 + w      (c_hi in {0,1}, w in 0..63)
  free      f = t*194 + 1 + (c_local%4)*48 + h
  f = t*194 and t*194+193 are zero guard columns.
Weight planes (host-derived, fp16, w-major compact [64, 4*5*48]):
  k=0: wxz   (* u[w+1] via DMA shift)    k=1: wxzUP (* u[w-1])
  k=2: wyzUP (* u[f-1] via offset view)  k=3: wyz   (* u[f+1])
  k=4: diag = 1 + wxz + wxzUP + wyz + wyzUP
Boundary weights are zeroed on host, so shift wrap-around terms vanish.
"""

import numpy as np
import sys

sys.path.insert(0, "/opt/trn_rl_repo")

import jax

jax.config.update("jax_compilation_cache_dir", "/tmp/jax_pcc")
jax.config.update("jax_persistent_cache_min_compile_time_secs", 0)
jax.config.update("jax_persistent_cache_min_entry_size_bytes", -1)

import concourse.bass as bass
from concourse import mybir
from concourse.bass_utils import run_bass_kernel_spmd

B, C, H, W = 4, 16, 48, 64
T = 8                 # tiles (b, half)
FD = 194              # per-tile free extent incl. 2 guards
FDA = 192             # active free size
WID = T * FD          # 1552
NPL = 5               # weight planes
WCOL = NPL * B * H // 2   # 480 weight cols appended per input row
CIN = T * FDA + WCOL  # 2016
LAM_MAX = 9.0         # Gershgorin bound: 1 + 2*4*max(w), w<1
N_ITER = 12

F32 = mybir.dt.float32
F16 = mybir.dt.float16

_COMPILED = {}
_EXEC_CACHE = {}


def _planes(wx, wy):
    """Per-batch [48,64] (h,w) planes in multiplication order
    (wxz, wxzUP, wyzUP, wyz, diag), boundaries zeroed."""
    wxz = wx.copy()
    wxz[:, -1] = 0.0
    wyz = wy.copy()
    wyz[-1, :] = 0.0
    wxzUP = np.zeros_like(wxz)
    wxzUP[:, 1:] = wxz[:, :-1]
    wyzUP = np.zeros_like(wyz)
    wyzUP[1:, :] = wyz[:-1, :]
    diag = 1.0 + wxz + wxzUP + wyz + wyzUP
    return wxz, wxzUP, wyzUP, wyz, diag


def _b2tile(ae_chans):
    """[8,48,64] -> [128,192] (p=(c_hi,w), f=(c_lo,h))."""
    a = ae_chans.reshape(2, 4, H, W)
    a = np.transpose(a, (0, 3, 1, 2))  # [c_hi, w, c_lo, h]
    return a.reshape(128, FDA)


def _tile2out(xt):
    """[128,192] -> [8,48,64]."""
    a = xt.reshape(2, W, 4, H)
    a = np.transpose(a, (0, 2, 3, 1))  # [c_hi, c_lo, h, w]
    return a.reshape(C // 2, H, W)


def _cheby_coeffs(lam_max, n_iter):
    """Per-iteration (gamma_k, c_next_k) for the scaled-direction Chebyshev
    recurrence: x += gamma_k*u ; r -= gamma_k*A u ; u = c_{k+1}*u + r."""
    lmin = 1.0
    theta = (lam_max + lmin) / 2.0
    delta = (lam_max - lmin) / 2.0
    sigma1 = theta / delta
    gammas, cnexts = [], []
    gamma = 1.0 / theta
    rho = 1.0 / sigma1
    for _ in range(n_iter):
        rho_next = 1.0 / (2.0 * sigma1 - rho)
        c_next = rho * gamma * delta / 2.0
        gamma_next = 2.0 * rho_next / delta
        gammas.append(gamma)
        cnexts.append(c_next)
        rho, gamma = rho_next, gamma_next
    return gammas, cnexts


def _build(n_iter):
    """Raw Bass program, single core, GPSIMD (DMA) + Vector engines only.
    Every instruction carries at most one wait (walrus codegen limit)."""
    nc = bass.Bass("TRN2", target_bir_lowering=False, debug=False,
                   num_devices=1, detect_race_conditions=False)
    cin_d = nc.dram_tensor("cin", [128, CIN], F16, kind="ExternalInput").ap()
    xo_d = nc.dram_tensor("xo", [128, T * FDA], F16,
                          kind="ExternalOutput").ap()

    gammas, cnexts = _cheby_coeffs(LAM_MAX, n_iter)
    theta = (LAM_MAX + 1.0) / 2.0

    s_bt = nc.alloc_sbuf_tensor("s_bt", [128, T * FDA], F16).ap()
    s_wc = nc.alloc_sbuf_tensor("s_wc", [64, 2 * WCOL], F16).ap()
    s_xo = nc.alloc_sbuf_tensor("s_xo", [128, T * FDA], F16).ap()
    wpl = nc.alloc_sbuf_tensor("wpl", [128, NPL * WID], F32).ap()
    u = nc.alloc_sbuf_tensor("u", [128, WID], F32).ap()
    r = nc.alloc_sbuf_tensor("r", [128, WID], F32).ap()
    x = nc.alloc_sbuf_tensor("x", [128, WID], F32).ap()
    uup = nc.alloc_sbuf_tensor("uup", [128, WID], F32).ap()
    udn = nc.alloc_sbuf_tensor("udn", [128, WID], F32).ap()
    pd = nc.alloc_sbuf_tensor("pd", [128, WID], F32).ap()
    p0 = nc.alloc_sbuf_tensor("p0", [128, WID], F32).ap()
    p1 = nc.alloc_sbuf_tensor("p1", [128, WID], F32).ap()
    p2 = nc.alloc_sbuf_tensor("p2", [128, WID], F32).ap()
    p3 = nc.alloc_sbuf_tensor("p3", [128, WID], F32).ap()

    w0 = wpl[:, 0 * WID:1 * WID]
    w1 = wpl[:, 1 * WID:2 * WID]
    w2 = wpl[:, 2 * WID:3 * WID]
    w3 = wpl[:, 3 * WID:4 * WID]
    w4 = wpl[:, 4 * WID:5 * WID]

    dsem = nc.alloc_semaphore("dsem")   # input/output + wpl-dup DMA
    ssem = nc.alloc_semaphore("ssem")   # per-iter shift DMAs
    vsem = nc.alloc_semaphore("vsem")   # wpl rows 0:64 built
    usem = nc.alloc_semaphore("usem")   # u-ready count
    osem = nc.alloc_semaphore("osem")   # s_xo cast done

    MULT = mybir.AluOpType.mult
    ADD = mybir.AluOpType.add

    with nc.Block() as block:

        @block.gpsimd
        def _(gp):
            gp.dma_start(s_bt, cin_d[:, 0:T * FDA]).then_inc(dsem, 16)
            gp.dma_start(s_wc[:, 0:WCOL],
                         cin_d[0:64, T * FDA:CIN]).then_inc(dsem, 16)
            gp.dma_start(s_wc[:, WCOL:2 * WCOL],
                         cin_d[64:128, T * FDA:CIN]).then_inc(dsem, 16)
            gp.wait_ge(vsem, 1)
            gp.dma_start(wpl[64:128, :], wpl[0:64, :]).then_inc(dsem, 16)
            for k in range(n_iter - 1):
                gp.wait_ge(usem, k + 1)
                gp.dma_start(uup[0:127, :], u[1:128, :]).then_inc(ssem, 16)
                gp.dma_start(udn[1:128, :], u[0:127, :]).then_inc(ssem, 16)
            gp.wait_ge(osem, 1)
            gp.dma_start(xo_d, s_xo).then_inc(dsem, 16)
            gp.wait_ge(dsem, 80)

        @block.vector
        def _(v):
            v.memset(uup, 0.0)
            v.memset(udn, 0.0)
            v.memset(p2, 0.0)
            v.memset(p3, 0.0)
            v.memset(r, 0.0)
            v.memset(wpl[0:64, :], 0.0)
            v.wait_ge(dsem, 48)  # all inputs in SBUF
            # scatter compact fp16 planes into guarded fp32 layout, rows 0:64
            scatter = []
            for k in range(NPL):
                for b in range(B):
                    src = s_wc[:, (b * NPL + k) * H:(b * NPL + k + 1) * H]
                    for half in range(2):
                        t = 2 * b + half
                        for cl in range(4):
                            off = k * WID + t * FD + 1 + cl * H
                            scatter.append(
                                v.tensor_copy(wpl[0:64, off:off + H], src))
            scatter[-1].then_inc(vsem, 1)
            # rhs placement: r active slices <- s_bt (fp16->fp32)
            for t in range(T):
                v.tensor_copy(r[:, t * FD + 1:t * FD + 193],
                              s_bt[:, t * FDA:(t + 1) * FDA])
            v.tensor_scalar_mul(u, r, 1.0 / theta).then_inc(usem, 1)
            v.wait_ge(dsem, 64)  # wpl rows 64:128 duplicated
            for k in range(n_iter):
                g = float(gammas[k])
                if k == 0:
                    v.tensor_scalar_mul(x, u, g)
                else:
                    v.scalar_tensor_tensor(x, u, g, x, MULT, ADD)
                if k == n_iter - 1:
                    break
                c = float(cnexts[k])
                v.tensor_tensor(pd, w4, u, MULT)
                v.tensor_tensor(p2[:, 1:WID], w2[:, 1:WID],
                                u[:, 0:WID - 1], MULT)
                v.tensor_tensor(p3[:, 0:WID - 1], w3[:, 0:WID - 1],
                                u[:, 1:WID], MULT)
                v.wait_ge(ssem, 32 * (k + 1))
                v.tensor_tensor(p0, w0, uup, MULT)
                v.tensor_tensor(p1, w1, udn, MULT)
                v.scalar_tensor_tensor(r, pd, -g, r, MULT, ADD)
                v.scalar_tensor_tensor(r, p0, g, r, MULT, ADD)
                v.scalar_tensor_tensor(r, p1, g, r, MULT, ADD)
                v.scalar_tensor_tensor(r, p2, g, r, MULT, ADD)
                v.scalar_tensor_tensor(r, p3, g, r, MULT, ADD)
                v.scalar_tensor_tensor(u, u, c, r, MULT, ADD).then_inc(
                    usem, 1)
            for t in range(T):
                cp = v.tensor_copy(s_xo[:, t * FDA:(t + 1) * FDA],
                                   x[:, t * FD + 1:t * FD + 193])
                if t == T - 1:
                    cp.then_inc(osem, 1)

    return nc


def _host_prep(ae, wxwy):
    cin = np.empty((128, CIN), dtype=np.float16)
    for t in range(T):
        b, half = t // 2, t % 2
        cin[:, t * FDA:(t + 1) * FDA] = _b2tile(
            ae[b, half * (C // 2):(half + 1) * (C // 2)]).astype(np.float16)
    # weight planes, w-major [64, 4*5*48], split across the two row halves
    wc = np.empty((64, B * NPL * H), dtype=np.float16)
    for b in range(B):
        planes = _planes(wxwy[b, 0], wxwy[b, 1])
        for k in range(NPL):
            # [48,64] (h,w) -> w-major [64,48]
            wc[:, (b * NPL + k) * H:(b * NPL + k + 1) * H] = \
                planes[k].T.astype(np.float16)
    cin[0:64, T * FDA:CIN] = wc[:, 0:WCOL]
    cin[64:128, T * FDA:CIN] = wc[:, WCOL:2 * WCOL]
    return cin


def _make_exec(nc):
    """Process-cached jit of the same bass_exec custom-call binding that
    bass_utils.run_bass_kernel_spmd / bass2jax.run_bass_via_pjrt uses for
    n_cores=1.  Re-jitting per call would re-load the NEFF on the device
    (~70 ms); this keeps one loaded executable alive."""
    from concourse.bass2jax import (_bass_exec_p, install_neuronx_cc_hook,
                                    partition_id_tensor)
    install_neuronx_cc_hook()
    partition_name = (nc.partition_id_tensor.name
                      if nc.partition_id_tensor else None)
    in_names, out_names, out_avals, out_shapes = [], [], [], []
    for alloc in nc.m.functions[0].allocations:
        if not isinstance(alloc, mybir.MemoryLocationSet):
            continue
        name = alloc.memorylocations[0].name
        if alloc.kind == "ExternalInput":
            if name != partition_name:
                in_names.append(name)
        elif alloc.kind == "ExternalOutput":
            out_names.append(name)
            shape = tuple(alloc.tensor_shape)
            dtype = mybir.dt.np(alloc.dtype)
            out_avals.append(jax.core.ShapedArray(shape, dtype))
            out_shapes.append((shape, dtype))
    n_params = len(in_names)
    all_names = in_names + out_names + (
        [partition_name] if partition_name else [])
    donate = tuple(range(n_params, n_params + len(out_names)))

    def _body(*args):
        operands = list(args)
        if partition_name:
            operands.append(partition_id_tensor())
        outs = _bass_exec_p.bind(
            *operands,
            out_avals=tuple(out_avals),
            in_names=tuple(all_names),
            out_names=tuple(out_names),
            lowering_input_output_aliases=(),
            sim_require_finite=True,
            sim_require_nnan=True,
            nc=nc,
        )
        return tuple(outs)

    fn = jax.jit(_body, donate_argnums=donate, keep_unused=True)
    return fn, in_names, out_names, out_shapes


def kernel(ae: np.ndarray, wxwy: np.ndarray) -> np.ndarray:
    ae = np.asarray(ae, dtype=np.float32)
    wxwy = np.asarray(wxwy, dtype=np.float32)

    cin = _host_prep(ae, wxwy)
    in_maps = [{"cin": cin}]

    if N_ITER not in _COMPILED:
        _COMPILED[N_ITER] = _build(N_ITER)
    nc = _COMPILED[N_ITER]

    global _LAST_BUILD
    _LAST_BUILD = (nc, in_maps)

    if N_ITER not in _EXEC_CACHE:
        # first call: compile + run through the sanctioned bass_utils path
        # (warms the NEFF/persistent caches), then build the cached
        # executable for repeat calls.
        res = run_bass_kernel_spmd(nc, in_maps, [0])
        _EXEC_CACHE[N_ITER] = _make_exec(nc)
        xo = np.asarray(res.results[0]["xo"], dtype=np.float32)
    else:
        fn, in_names, out_names, out_shapes = _EXEC_CACHE[N_ITER]
        zeros = [np.zeros(shape, dtype) for shape, dtype in out_shapes]
        out_arrs = fn(*[in_maps[0][n] for n in in_names], *zeros)
        xo = np.asarray(out_arrs[out_names.index("xo")], dtype=np.float32)

    out = np.empty((B, C, H, W), dtype=np.float32)
    for t in range(T):
        b, half = t // 2, t % 2
        out[b, half * (C // 2):(half + 1) * (C // 2)] = _tile2out(
            xo[:, t * FDA:(t + 1) * FDA])
    return out


NCORE = 1  # cores used by _LAST_BUILD (test.py reads this)


# revision 8
# speedup vs baseline: 4.9333x; 1.0277x over previous
"""Trainium2 Bass kernel for GridSmoother: per-batch SPD grid-Laplacian solve.

System: L = I + Dx^T Wx Dx + Dy^T Wy Dy over a 48x64 grid, 16 channels per
batch, B=4.  lambda(L) in [1, 9] (Gershgorin, weights < 1), so a
fixed-coefficient Chebyshev iteration on the 5-point stencil converges at
~0.5x error per iteration; K=12 iterations reach ~5e-4 relative error,
far inside the 2e-2 gate.

This problem is wall-clock-dominated by host->device dispatch through the
PJRT relay, not device compute (~0.3 ms of simulated device time).  The
kernel is therefore built to minimize per-call overhead:
  * single NeuronCore (core count showed no win at fixed volume, and the
    1-core jit path skips shard_map),
  * fp16 I/O, one merged input buffer (ae rhs + compact weight planes,
    516KB) and one fp16 output (393KB),
  * no TensorEngine/PSUM: horizontal (w+-1) neighbor terms use
    partition-shifted SBUF->SBUF DMA copies of u; vertical (h+-1) terms
    use free-dim offset views; everything else is Vector-engine ops,
  * a persistent jax compilation cache plus a process-level cache of the
    loaded executable: re-creating the PJRT executable per call costs
    ~70 ms of NEFF reload on the device, so kernel() compiles/loads via
    bass_utils.run_bass_kernel_spmd on the first call and executes the
    cached executable (same custom-call binding) on repeat calls,
  * fixed lam_max=9.0 so the compiled program is input-independent.

Tile layout (8 tiles t = 2*b + half, half selects 8 of 16 channels):
  partition p = (c_local//4)*64 + w      (c_hi in {0,1}, w in 0..63)
  free      f = t*194 + 1 + (c_local%4)*48 + h
  f = t*194 and t*194+193 are zero guard columns.
Weight planes (host-derived, fp16, w-major compact [64, 4*5*48]):
  k=0: wxz   (* u[w+1] via DMA shift)    k=1: wxzUP (* u[w-1])
  k=2: wyzUP (* u[f-1] via offset view)  k=3: wyz   (* u[f+1])
  k=4: diag = 1 + wxz + wxzUP + wyz + wyzUP
Boundary weights are zeroed on host, so shift wrap-around terms vanish.
"""

import numpy as np
import sys

sys.path.insert(0, "/opt/trn_rl_repo")

import jax

jax.config.update("jax_compilation_cache_dir", "/tmp/jax_pcc")
jax.config.update("jax_persistent_cache_min_compile_time_secs", 0)
jax.config.update("jax_persistent_cache_min_entry_size_bytes", -1)

import concourse.bass as bass
from concourse import mybir
from concourse.bass_utils import run_bass_kernel_spmd

B, C, H, W = 4, 16, 48, 64
T = 8                 # tiles (b, half)
FD = 194              # per-tile free extent incl. 2 guards
FDA = 192             # active free size
WID = T * FD          # 1552
NPL = 5               # weight planes
WCOL = NPL * B * H // 2   # 480 weight cols appended per input row
CIN = T * FDA + WCOL  # 2016
LAM_MAX = 9.0         # Gershgorin bound: 1 + 2*4*max(w), w<1
N_ITER = 12

F32 = mybir.dt.float32
F16 = mybir.dt.float16

_COMPILED = {}
_EXEC_CACHE = {}


def _planes(wx, wy):
    """Per-batch [48,64] (h,w) planes in multiplication order
    (wxz, wxzUP, wyzUP, wyz, diag), boundaries zeroed."""
    wxz = wx.copy()
    wxz[:, -1] = 0.0
    wyz = wy.copy()
    wyz[-1, :] = 0.0
    wxzUP = np.zeros_like(wxz)
    wxzUP[:, 1:] = wxz[:, :-1]
    wyzUP = np.zeros_like(wyz)
    wyzUP[1:, :] = wyz[:-1, :]
    diag = 1.0 + wxz + wxzUP + wyz + wyzUP
    return wxz, wxzUP, wyzUP, wyz, diag


def _b2tile(ae_chans):
    """[8,48,64] -> [128,192] (p=(c_hi,w), f=(c_lo,h))."""
    a = ae_chans.reshape(2, 4, H, W)
    a = np.transpose(a, (0, 3, 1, 2))  # [c_hi, w, c_lo, h]
    return a.reshape(128, FDA)


def _tile2out(xt):
    """[128,192] -> [8,48,64]."""
    a = xt.reshape(2, W, 4, H)
    a = np.transpose(a, (0, 2, 3, 1))  # [c_hi, c_lo, h, w]
    return a.reshape(C // 2, H, W)


def _cheby_coeffs(lam_max, n_iter):
    """Per-iteration (gamma_k, c_next_k) for the scaled-direction Chebyshev
    recurrence: x += gamma_k*u ; r -= gamma_k*A u ; u = c_{k+1}*u + r."""
    lmin = 1.0
    theta = (lam_max + lmin) / 2.0
    delta = (lam_max - lmin) / 2.0
    sigma1 = theta / delta
    gammas, cnexts = [], []
    gamma = 1.0 / theta
    rho = 1.0 / sigma1
    for _ in range(n_iter):
        rho_next = 1.0 / (2.0 * sigma1 - rho)
        c_next = rho * gamma * delta / 2.0
        gamma_next = 2.0 * rho_next / delta
        gammas.append(gamma)
        cnexts.append(c_next)
        rho, gamma = rho_next, gamma_next
    return gammas, cnexts


def _build(n_iter):
    """Raw Bass program, single core, GPSIMD (DMA) + Vector engines only.
    Every instruction carries at most one wait (walrus codegen limit)."""
    nc = bass.Bass("TRN2", target_bir_lowering=False, debug=False,
                   num_devices=1, detect_race_conditions=False)
    cin_d = nc.dram_tensor("cin", [128, CIN], F16, kind="ExternalInput").ap()
    xo_d = nc.dram_tensor("xo", [128, T * FDA], F16,
                          kind="ExternalOutput").ap()

    gammas, cnexts = _cheby_coeffs(LAM_MAX, n_iter)
    theta = (LAM_MAX + 1.0) / 2.0

    s_bt = nc.alloc_sbuf_tensor("s_bt", [128, T * FDA], F16).ap()
    s_wc = nc.alloc_sbuf_tensor("s_wc", [64, 2 * WCOL], F16).ap()
    s_xo = nc.alloc_sbuf_tensor("s_xo", [128, T * FDA], F16).ap()
    wpl = nc.alloc_sbuf_tensor("wpl", [128, NPL * WID], F32).ap()
    u = nc.alloc_sbuf_tensor("u", [128, WID], F32).ap()
    r = nc.alloc_sbuf_tensor("r", [128, WID], F32).ap()
    x = nc.alloc_sbuf_tensor("x", [128, WID], F32).ap()
    uup = nc.alloc_sbuf_tensor("uup", [128, WID], F32).ap()
    udn = nc.alloc_sbuf_tensor("udn", [128, WID], F32).ap()
    pd = nc.alloc_sbuf_tensor("pd", [128, WID], F32).ap()
    p0 = nc.alloc_sbuf_tensor("p0", [128, WID], F32).ap()
    p1 = nc.alloc_sbuf_tensor("p1", [128, WID], F32).ap()
    p2 = nc.alloc_sbuf_tensor("p2", [128, WID], F32).ap()
    p3 = nc.alloc_sbuf_tensor("p3", [128, WID], F32).ap()

    w0 = wpl[:, 0 * WID:1 * WID]
    w1 = wpl[:, 1 * WID:2 * WID]
    w2 = wpl[:, 2 * WID:3 * WID]
    w3 = wpl[:, 3 * WID:4 * WID]
    w4 = wpl[:, 4 * WID:5 * WID]

    dsem = nc.alloc_semaphore("dsem")   # input/output + wpl-dup DMA
    ssem = nc.alloc_semaphore("ssem")   # per-iter shift DMAs
    vsem = nc.alloc_semaphore("vsem")   # wpl rows 0:64 built
    usem = nc.alloc_semaphore("usem")   # u-ready count
    osem = nc.alloc_semaphore("osem")   # s_xo cast done

    MULT = mybir.AluOpType.mult
    ADD = mybir.AluOpType.add

    with nc.Block() as block:

        @block.gpsimd
        def _(gp):
            gp.dma_start(s_bt, cin_d[:, 0:T * FDA]).then_inc(dsem, 16)
            gp.dma_start(s_wc[:, 0:WCOL],
                         cin_d[0:64, T * FDA:CIN]).then_inc(dsem, 16)
            gp.dma_start(s_wc[:, WCOL:2 * WCOL],
                         cin_d[64:128, T * FDA:CIN]).then_inc(dsem, 16)
            gp.wait_ge(vsem, 1)
            gp.dma_start(wpl[64:128, :], wpl[0:64, :]).then_inc(dsem, 16)
            for k in range(n_iter - 1):
                gp.wait_ge(usem, k + 1)
                gp.dma_start(uup[0:127, :], u[1:128, :]).then_inc(ssem, 16)
                gp.dma_start(udn[1:128, :], u[0:127, :]).then_inc(ssem, 16)
            gp.wait_ge(osem, 1)
            gp.dma_start(xo_d, s_xo).then_inc(dsem, 16)
            gp.wait_ge(dsem, 80)

        @block.vector
        def _(v):
            v.memset(uup, 0.0)
            v.memset(udn, 0.0)
            v.memset(p2, 0.0)
            v.memset(p3, 0.0)
            v.memset(r, 0.0)
            v.memset(wpl[0:64, :], 0.0)
            v.wait_ge(dsem, 48)  # all inputs in SBUF
            # scatter compact fp16 planes into guarded fp32 layout, rows 0:64
            scatter = []
            for k in range(NPL):
                for b in range(B):
                    src = s_wc[:, (b * NPL + k) * H:(b * NPL + k + 1) * H]
                    for half in range(2):
                        t = 2 * b + half
                        for cl in range(4):
                            off = k * WID + t * FD + 1 + cl * H
                            scatter.append(
                                v.tensor_copy(wpl[0:64, off:off + H], src))
            scatter[-1].then_inc(vsem, 1)
            # rhs placement: r active slices <- s_bt (fp16->fp32)
            for t in range(T):
                v.tensor_copy(r[:, t * FD + 1:t * FD + 193],
                              s_bt[:, t * FDA:(t + 1) * FDA])
            v.tensor_scalar_mul(u, r, 1.0 / theta).then_inc(usem, 1)
            v.wait_ge(dsem, 64)  # wpl rows 64:128 duplicated
            for k in range(n_iter):
                g = float(gammas[k])
                if k == 0:
                    v.tensor_scalar_mul(x, u, g)
                else:
                    v.scalar_tensor_tensor(x, u, g, x, MULT, ADD)
                if k == n_iter - 1:
                    break
                c = float(cnexts[k])
                v.tensor_tensor(pd, w4, u, MULT)
                v.tensor_tensor(p2[:, 1:WID], w2[:, 1:WID],
                                u[:, 0:WID - 1], MULT)
                v.tensor_tensor(p3[:, 0:WID - 1], w3[:, 0:WID - 1],
                                u[:, 1:WID], MULT)
                v.wait_ge(ssem, 32 * (k + 1))
                v.tensor_tensor(p0, w0, uup, MULT)
                v.tensor_tensor(p1, w1, udn, MULT)
                v.scalar_tensor_tensor(r, pd, -g, r, MULT, ADD)
                v.scalar_tensor_tensor(r, p0, g, r, MULT, ADD)
                v.scalar_tensor_tensor(r, p1, g, r, MULT, ADD)
                v.scalar_tensor_tensor(r, p2, g, r, MULT, ADD)
                v.scalar_tensor_tensor(r, p3, g, r, MULT, ADD)
                v.scalar_tensor_tensor(u, u, c, r, MULT, ADD).then_inc(
                    usem, 1)
            for t in range(T):
                cp = v.tensor_copy(s_xo[:, t * FDA:(t + 1) * FDA],
                                   x[:, t * FD + 1:t * FD + 193])
                if t == T - 1:
                    cp.then_inc(osem, 1)

    return nc


def _host_prep(ae, wxwy):
    cin = np.empty((128, CIN), dtype=np.float16)
    # rhs: ae [b, (half,c_hi,c_lo) chan, h, w] -> [(c_hi,w) part, (b,half,c_lo,h)]
    a = ae.reshape(B, 2, 2, 4, H, W)           # b, half, c_hi, c_lo, h, w
    a = a.transpose(2, 5, 0, 1, 3, 4)          # c_hi, w, b, half, c_lo, h
    cin[:, 0:T * FDA] = a.reshape(128, T * FDA).astype(np.float16)
    # weight planes, w-major [64, (b,plane,h)], split across the row halves
    wx = wxwy[:, 0].copy()
    wy = wxwy[:, 1].copy()
    wx[:, :, -1] = 0.0
    wy[:, -1, :] = 0.0
    wxUP = np.zeros_like(wx)
    wxUP[:, :, 1:] = wx[:, :, :-1]
    wyUP = np.zeros_like(wy)
    wyUP[:, 1:, :] = wy[:, :-1, :]
    diag = 1.0 + wx + wxUP + wy + wyUP
    planes = np.stack([wx, wxUP, wyUP, wy, diag], axis=1)  # [B,5,H,W]
    wc = planes.transpose(3, 0, 1, 2).reshape(W, B * NPL * H)  # w-major
    wc = wc.astype(np.float16)
    cin[0:64, T * FDA:CIN] = wc[:, 0:WCOL]
    cin[64:128, T * FDA:CIN] = wc[:, WCOL:2 * WCOL]
    return cin


def _make_exec(nc):
    """Process-cached jit of the same bass_exec custom-call binding that
    bass_utils.run_bass_kernel_spmd / bass2jax.run_bass_via_pjrt uses for
    n_cores=1.  Re-jitting per call would re-load the NEFF on the device
    (~70 ms); this keeps one loaded executable alive."""
    from concourse.bass2jax import (_bass_exec_p, install_neuronx_cc_hook,
                                    partition_id_tensor)
    install_neuronx_cc_hook()
    partition_name = (nc.partition_id_tensor.name
                      if nc.partition_id_tensor else None)
    in_names, out_names, out_avals, out_shapes = [], [], [], []
    for alloc in nc.m.functions[0].allocations:
        if not isinstance(alloc, mybir.MemoryLocationSet):
            continue
        name = alloc.memorylocations[0].name
        if alloc.kind == "ExternalInput":
            if name != partition_name:
                in_names.append(name)
        elif alloc.kind == "ExternalOutput":
            out_names.append(name)
            shape = tuple(alloc.tensor_shape)
            dtype = mybir.dt.np(alloc.dtype)
            out_avals.append(jax.core.ShapedArray(shape, dtype))
            out_shapes.append((shape, dtype))
    # No donated zero output buffers: run_bass_via_pjrt donates zeros so
    # kernels that only partially write their outputs stay deterministic,
    # but this program DMA-writes every byte of xo, and the zeros would
    # cost an extra 393KB host->device transfer per call.
    all_names = in_names + (
        [partition_name] if partition_name else [])

    def _body(*args):
        operands = list(args)
        if partition_name:
            operands.append(partition_id_tensor())
        outs = _bass_exec_p.bind(
            *operands,
            out_avals=tuple(out_avals),
            in_names=tuple(all_names),
            out_names=tuple(out_names),
            lowering_input_output_aliases=(),
            sim_require_finite=True,
            sim_require_nnan=True,
            nc=nc,
        )
        return tuple(outs)

    fn = jax.jit(_body, keep_unused=True)
    return fn, in_names, out_names, out_shapes


def kernel(ae: np.ndarray, wxwy: np.ndarray) -> np.ndarray:
    ae = np.asarray(ae, dtype=np.float32)
    wxwy = np.asarray(wxwy, dtype=np.float32)

    cin = _host_prep(ae, wxwy)
    in_maps = [{"cin": cin}]

    if N_ITER not in _COMPILED:
        _COMPILED[N_ITER] = _build(N_ITER)
    nc = _COMPILED[N_ITER]

    global _LAST_BUILD
    _LAST_BUILD = (nc, in_maps)

    if N_ITER not in _EXEC_CACHE:
        # first call: compile + run through the sanctioned bass_utils path
        # (warms the NEFF/persistent caches), then build and warm the
        # cached executable used by all subsequent calls.
        run_bass_kernel_spmd(nc, in_maps, [0])
        _EXEC_CACHE[N_ITER] = _make_exec(nc)
    fn, in_names, out_names, out_shapes = _EXEC_CACHE[N_ITER]
    out_arrs = fn(*[in_maps[0][n] for n in in_names])
    xo = np.asarray(out_arrs[out_names.index("xo")], dtype=np.float32)

    # [(c_hi,w), (b,half,c_lo,h)] -> [b, chan, h, w]
    xr = xo.reshape(2, W, B, 2, 4, H)
    out = xr.transpose(2, 3, 0, 4, 5, 1).reshape(B, C, H, W)
    return np.ascontiguousarray(out)


NCORE = 1  # cores used by _LAST_BUILD (test.py reads this)
